# revision 2
# baseline (speedup 1.0000x reference)
"""Causal self-attention (single head) on 8 trn2 NeuronCores — fp8 edition.

Full inputs:  x [4, 4096, 1024] f32, Wq/Wk/Wv [1024, 1024] f32.
Output:       [4, 4096, 1024] f32 = softmax(causal(q k^T / sqrt(d))) @ v.

Sharding: 2 cores per batch; query tiles split qi%4 in {0,3} vs {1,2}
(exactly balanced causal work), slot j covers P=2j+2 key tiles.

v2: all matmuls in fp8-e4m3 with DoubleRow perf mode (2 k-tiles of
contraction per pass, ~4x bf16 throughput measured), EXCEPT slot 0
(queries attending to <=256 keys) which runs in bf16: absolute output
error for rows with few attended keys is dominated by input
quantization noise that does not average out, so slot 0 keeps bf16
inputs end-to-end.

Scores are computed TRANSPOSED (sT[k, q] per 128x128 k-tile block):
lhsT = kT-block, rhs = qT-slot. exp() then writes attnT directly (no
transposes on the attention path). Causal masking is multiplicative
(0/1) on attnT after exp. Row sums (needed for softmax normalization)
come from a tiny extra matmul per k-tile pair: lhsT=attnT, rhs=ones/64
-> psum [128q, 1] accumulating rowsum/64; reciprocal gives 64/rowsum,
z is scaled by 1/64 into fp8 (avoids e4m3 overflow: |z| can reach
~300 > 240 = e4m3 max), and the two scales cancel exactly in the
output projection's recip-scaled copy.

PV is reassociated: out = ((attn @ x) @ Wv) / rowsum, so V is never
materialized. q/k magnitudes stay natural (std ~0.58, e4m3 sweet
spot); the 1/sqrt(d) softmax scale is folded into the Exp activation.
"""

import math
import os
import numpy as np
from contextlib import ExitStack

import concourse.bass as bass
import concourse.tile as tile
from concourse import bacc, mybir
from concourse.bass_utils import run_bass_kernel_spmd

F32 = mybir.dt.float32
BF16 = mybir.dt.bfloat16
FP8 = mybir.dt.float8e4
DR = mybir.MatmulPerfMode.DoubleRow

B = 4
S = 4096
D = 1024
N_CORES = 8
ZSCALE = 1.0 / 64.0          # keeps |z| and rowsum within fp8/clean f32


def core_slot_tiles(h: int) -> list[int]:
    """Query-tile index (qi) handled in slot j, for core half h."""
    out = []
    for m in range(8):
        if h == 0:
            out += [4 * m, 4 * m + 3]
        else:
            out += [4 * m + 1, 4 * m + 2]
    return out


def build_masksT(tiles: list[int]) -> np.ndarray:
    """[n_slots, 2, 128, 128] multiplicative masks (k-major) for the last
    two k-tiles of each slot: m[t, k, q] = 1 if key_global <= query_global."""
    n_slots = len(tiles)
    masks = np.zeros((n_slots, 2, 128, 128), dtype=np.float32)
    k = np.arange(128)[:, None]
    q = np.arange(128)[None, :]
    for j, qi in enumerate(tiles):
        P = 2 * j + 2
        for t in range(2):
            kt = P - 2 + t
            valid = (128 * kt + k) <= (128 * qi + q)
            masks[j, t] = valid.astype(np.float32)
    return masks


def build_attention_program(nc, loop_n: int = 1):
    s_tiles = S // 128              # 32
    n_slots = s_tiles // 2          # 16
    d = D
    dk = d // 128                   # 8 contraction chunks of 128
    dn = d // 512                   # 2 output column blocks
    sq = n_slots * 128              # 2048 own query rows
    scale = 1.0 / math.sqrt(float(d))

    x8_in = nc.dram_tensor("x8", [S, d], FP8, kind="ExternalInput")
    xb16_in = nc.dram_tensor("xb16", [256, d], BF16, kind="ExternalInput")
    xT8_in = nc.dram_tensor("xT8", [d, S], FP8, kind="ExternalInput")
    xT16_in = nc.dram_tensor("xT16", [d, 256], BF16, kind="ExternalInput")
    xqT8_in = nc.dram_tensor("xqT8", [d, sq], FP8, kind="ExternalInput")
    xq0T16_in = nc.dram_tensor("xq0T16", [d, 128], BF16, kind="ExternalInput")
    wq8_in = nc.dram_tensor("Wq8", [d, d], FP8, kind="ExternalInput")
    wk8_in = nc.dram_tensor("Wk8", [d, d], FP8, kind="ExternalInput")
    wv8_in = nc.dram_tensor("Wv8", [d, d], FP8, kind="ExternalInput")
    wq16_in = nc.dram_tensor("Wq16", [d, d], BF16, kind="ExternalInput")
    wk16_in = nc.dram_tensor("Wk16", [d, d], BF16, kind="ExternalInput")
    wv16_in = nc.dram_tensor("Wv16", [d, d], BF16, kind="ExternalInput")
    masksT8_in = nc.dram_tensor("masksT8", [n_slots, 2, 128, 128], FP8,
                                kind="ExternalInput")
    masks016_in = nc.dram_tensor("masks016", [2, 128, 128], BF16,
                                 kind="ExternalInput")
    out_dram = nc.dram_tensor("out", [n_slots, 128, d], F32,
                              kind="ExternalOutput")

    x8_r = x8_in.ap().rearrange("(t p) d -> p t d", p=128)     # [128,32,1024]
    xb16_r = xb16_in.ap().rearrange("(t p) d -> p t d", p=128)  # [128,2,1024]
    xT8_r = xT8_in.ap().rearrange("(do p) s -> p do s", p=128)
    xT16_r = xT16_in.ap().rearrange("(do p) s -> p do s", p=128)
    xqT8_r = xqT8_in.ap().rearrange("(do p) s -> p do s", p=128)
    xq0T16_r = xq0T16_in.ap().rearrange("(do p) s -> p do s", p=128)
    wq8_r = wq8_in.ap().rearrange("(ko p) n -> p ko n", p=128)
    wk8_r = wk8_in.ap().rearrange("(ko p) n -> p ko n", p=128)
    wv8_r = wv8_in.ap().rearrange("(ko p) n -> p ko n", p=128)
    wq16_r = wq16_in.ap().rearrange("(ko p) n -> p ko n", p=128)
    wk16_r = wk16_in.ap().rearrange("(ko p) n -> p ko n", p=128)
    wv16_r = wv16_in.ap().rearrange("(ko p) n -> p ko n", p=128)
    masksT8_r = masksT8_in.ap().rearrange("j t k q -> k j t q")
    masks016_r = masks016_in.ap().rearrange("t k q -> k t q")

    with tile.TileContext(nc) as tc, ExitStack() as outer, \
         ExitStack() as ctx:
        if loop_n > 1:
            outer.enter_context(tc.For_i(0, loop_n, 1))
        res = ctx.enter_context(tc.tile_pool(name="res", bufs=1))
        x8_nat = res.tile([128, s_tiles, d], FP8)
        xb16_nat = res.tile([128, 2, d], BF16)
        kT8 = res.tile([128, dk, S], FP8)
        kT16 = res.tile([128, dk, 256], BF16)
        qT8 = res.tile([128, dk, sq], FP8)
        qT16 = res.tile([128, dk, 128], BF16)
        wv8_sb = res.tile([128, dk, d], FP8)
        wv16_sb = res.tile([128, dk, d], BF16)
        masksT8_sb = res.tile([128, n_slots, 2, 128], FP8)
        masks016_sb = res.tile([128, 2, 128], BF16)
        rs_w = 1
        ones8 = res.tile([128, 2, rs_w], FP8)
        ones16 = res.tile([128, 2, rs_w], BF16)
        nc.vector.memset(ones8, ZSCALE)
        nc.vector.memset(ones16, ZSCALE)

        def load_residents():
            """Stage-2 residents on the Activation HWDGE ring so stage-1
            input chunks on the SP ring are never stuck behind them."""
            eng = (nc.scalar if os.environ.get("V2_RES_ACT", "1") == "1"
                   else nc.sync)
            eng.dma_start(out=masks016_sb, in_=masks016_r)
            eng.dma_start(out=masksT8_sb, in_=masksT8_r)
            eng.dma_start(out=xb16_nat, in_=xb16_r)
            for piece in range(4):
                eng.dma_start(
                    out=x8_nat[:, 8 * piece:8 * (piece + 1), :],
                    in_=x8_r[:, 8 * piece:8 * (piece + 1), :])
            eng.dma_start(out=wv16_sb, in_=wv16_r)
            eng.dma_start(out=wv8_sb, in_=wv8_r)

        # ---------------- stage 1: projections ----------------
        stage1 = ExitStack()
        ps_proj = stage1.enter_context(
            tc.tile_pool(name="ps_proj", bufs=4, space="PSUM"))

        def copy_out(eng, dst, src):
            if eng is nc.scalar:
                nc.scalar.activation(
                    out=dst, in_=src,
                    func=mybir.ActivationFunctionType.Copy)
            else:
                eng.tensor_copy(dst, src)

        def proj8(dst, w_sb, src_r, n_chunks, copy_engines):
            """dst[:, dot, 512ci:+512] = W^T @ src for fp8 DoubleRow."""
            with tc.tile_pool(name="xt8", bufs=2) as xtp:
                for ci in range(n_chunks):
                    xc = xtp.tile([128, dk, 512], FP8, tag="x8c")
                    nc.sync.dma_start(
                        out=xc, in_=src_r[:, :, 512 * ci:512 * (ci + 1)])
                    for dot in range(dk):
                        ps = ps_proj.tile([128, 512], F32, tag="pp")
                        for ko in range(dk // 2):
                            nc.tensor.matmul(
                                ps,
                                w_sb[:, 2 * ko:2 * ko + 2,
                                     128 * dot:128 * (dot + 1)],
                                xc[:, 2 * ko:2 * ko + 2, :],
                                start=(ko == 0), stop=(ko == dk // 2 - 1),
                                perf_mode=DR)
                        eng = copy_engines[(ci * dk + dot) % len(copy_engines)]
                        copy_out(eng, dst[:, dot, 512 * ci:512 * (ci + 1)], ps)

        def proj16(dst, w_sb, src_r, width, copy_engines):
            """bf16 mini-projection of `width` columns (width <= 512)."""
            with tc.tile_pool(name="xt16", bufs=1) as xtp:
                xc = xtp.tile([128, dk, width], BF16, tag="x16c")
                nc.sync.dma_start(out=xc, in_=src_r)
                for dot in range(dk):
                    ps = ps_proj.tile([128, 512], F32, tag="pp")
                    for ko in range(dk):
                        nc.tensor.matmul(
                            ps[:, :width],
                            w_sb[:, ko, 128 * dot:128 * (dot + 1)],
                            xc[:, ko, :],
                            start=(ko == 0), stop=(ko == dk - 1))
                    eng = copy_engines[dot % len(copy_engines)]
                    copy_out(eng, dst[:, dot, :], ps[:, :width])

        both = [nc.vector, nc.scalar]
        with tc.tile_pool(name="wk", bufs=1) as wkp:
            wk8_sb = wkp.tile([128, dk, d], FP8, tag="w8")
            nc.sync.dma_start(out=wk8_sb, in_=wk8_r)
            proj8(kT8, wk8_sb, xT8_r, S // 512, both)
            wk16_sb = wkp.tile([128, dk, d], BF16, tag="w16")
            nc.sync.dma_start(out=wk16_sb, in_=wk16_r)
            proj16(kT16, wk16_sb, xT16_r, 256, both)
        with tc.tile_pool(name="wq", bufs=1) as wqp:
            wq8_sb = wqp.tile([128, dk, d], FP8, tag="w8")
            nc.sync.dma_start(out=wq8_sb, in_=wq8_r)
            proj8(qT8, wq8_sb, xqT8_r, sq // 512, both)
            wq16_sb = wqp.tile([128, dk, d], BF16, tag="w16")
            nc.sync.dma_start(out=wq16_sb, in_=wq16_r)
            proj16(qT16, wq16_sb, xq0T16_r, 128, both)
        if resmode != "top":
            load_residents()

        # ---------------- stage 2: attention ----------------
        stage1.close()
        resmode = "late"
        def load_residents():
            if resmode == "top":
                nc.sync.dma_start(out=x8_nat, in_=x8_r)
                nc.sync.dma_start(out=xb16_nat, in_=xb16_r)
                nc.sync.dma_start(out=wv8_sb, in_=wv8_r)
                nc.sync.dma_start(out=wv16_sb, in_=wv16_r)
                nc.sync.dma_start(out=masksT8_sb, in_=masksT8_r)
                nc.sync.dma_start(out=masks016_sb, in_=masks016_r)
            else:
                nc.sync.dma_start(out=masks016_sb, in_=masks016_r)
                nc.sync.dma_start(out=masksT8_sb, in_=masksT8_r)
                nc.sync.dma_start(out=xb16_nat, in_=xb16_r)
                for piece in range(4):
                    nc.sync.dma_start(
                        out=x8_nat[:, 8 * piece:8 * (piece + 1), :],
                        in_=x8_r[:, 8 * piece:8 * (piece + 1), :])
                nc.sync.dma_start(out=wv16_sb, in_=wv16_r)
                nc.sync.dma_start(out=wv8_sb, in_=wv8_r)
        if resmode == "top":
            load_residents()

        # ---------------- stage 1: projections ----------------
        stage1 = ExitStack()
        ps_proj = stage1.enter_context(
            tc.tile_pool(name="ps_proj", bufs=4, space="PSUM"))

        def copy_out(eng, dst, src):
            if eng is nc.scalar:
                nc.scalar.activation(
                    out=dst, in_=src,
                    func=mybir.ActivationFunctionType.Copy)
            else:
                eng.tensor_copy(dst, src)

        def proj8(dst, w_sb, src_r, n_chunks, copy_engines):
            """dst[:, dot, 512ci:+512] = W^T @ src for fp8 DoubleRow."""
            with tc.tile_pool(name="xt8", bufs=2) as xtp:
                for ci in range(n_chunks):
                    xc = xtp.tile([128, dk, 512], FP8, tag="x8c")
                    nc.sync.dma_start(
                        out=xc, in_=src_r[:, :, 512 * ci:512 * (ci + 1)])
                    for dot in range(dk):
                        ps = ps_proj.tile([128, 512], F32, tag="pp")
                        for ko in range(dk // 2):
                            nc.tensor.matmul(
                                ps,
                                w_sb[:, 2 * ko:2 * ko + 2,
                                     128 * dot:128 * (dot + 1)],
                                xc[:, 2 * ko:2 * ko + 2, :],
                                start=(ko == 0), stop=(ko == dk // 2 - 1),
                                perf_mode=DR)
                        eng = copy_engines[(ci * dk + dot) % len(copy_engines)]
                        copy_out(eng, dst[:, dot, 512 * ci:512 * (ci + 1)], ps)

        def proj16(dst, w_sb, src_r, width, copy_engines):
            """bf16 mini-projection of `width` columns (width <= 512)."""
            with tc.tile_pool(name="xt16", bufs=1) as xtp:
                xc = xtp.tile([128, dk, width], BF16, tag="x16c")
                nc.sync.dma_start(out=xc, in_=src_r)
                for dot in range(dk):
                    ps = ps_proj.tile([128, 512], F32, tag="pp")
                    for ko in range(dk):
                        nc.tensor.matmul(
                            ps[:, :width],
                            w_sb[:, ko, 128 * dot:128 * (dot + 1)],
                            xc[:, ko, :],
                            start=(ko == 0), stop=(ko == dk - 1))
                    eng = copy_engines[dot % len(copy_engines)]
                    copy_out(eng, dst[:, dot, :], ps[:, :width])

        both = [nc.vector, nc.scalar]
        with tc.tile_pool(name="wk", bufs=1) as wkp:
            wk8_sb = wkp.tile([128, dk, d], FP8, tag="w8")
            nc.sync.dma_start(out=wk8_sb, in_=wk8_r)
            proj8(kT8, wk8_sb, xT8_r, S // 512, both)
            wk16_sb = wkp.tile([128, dk, d], BF16, tag="w16")
            nc.sync.dma_start(out=wk16_sb, in_=wk16_r)
            proj16(kT16, wk16_sb, xT16_r, 256, both)
        with tc.tile_pool(name="wq", bufs=1) as wqp:
            wq8_sb = wqp.tile([128, dk, d], FP8, tag="w8")
            nc.sync.dma_start(out=wq8_sb, in_=wq8_r)
            proj8(qT8, wq8_sb, xqT8_r, sq // 512, both)
            wq16_sb = wqp.tile([128, dk, d], BF16, tag="w16")
            nc.sync.dma_start(out=wq16_sb, in_=wq16_r)
            proj16(qT16, wq16_sb, xq0T16_r, 128, both)
        if resmode != "top":
            load_residents()

        # ---------------- stage 2: attention ----------------
        stage1.close()
        ps_s = ctx.enter_context(
            tc.tile_pool(name="ps_s", bufs=3, space="PSUM"))
        ps_zp = ctx.enter_context(
            tc.tile_pool(name="ps_z", bufs=1, space="PSUM"))
        ps_op = ctx.enter_context(
            tc.tile_pool(name="ps_o", bufs=2, space="PSUM"))
        attnT_p = ctx.enter_context(tc.tile_pool(name="attnT_p", bufs=6))
        z_p = ctx.enter_context(tc.tile_pool(name="z_p", bufs=2))
        sm_p = ctx.enter_context(tc.tile_pool(name="sm_p", bufs=2))
        out_p = ctx.enter_context(tc.tile_pool(name="out_p", bufs=2))

        for j in range(n_slots):
            P = 2 * j + 2                    # k-tiles (128 keys each)
            nb = (P + 3) // 4                # psum blocks of 4 k-tiles
            fp8 = j >= 1
            ps_zr = ps_zp.tile([128, d + 512], F32, tag="z")
            ps_z = ps_zr[:, :d]
            ps_r = ps_zr[:, d:d + rs_w]

            attnTs = []
            # pass 1: scoresT -> exp -> (masked) attnT, per 4-k-tile block
            for g in range(nb):
                nt = 4 if (g < nb - 1 or P % 4 == 0) else P % 4
                ps = ps_s.tile([128, 512], F32, tag="ps")
                for t in range(nt):
                    kt = 4 * g + t
                    if fp8:
                        for eo in range(dk // 2):
                            nc.tensor.matmul(
                                ps[:, 128 * t:128 * (t + 1)],
                                kT8[:, 2 * eo:2 * eo + 2,
                                    128 * kt:128 * (kt + 1)],
                                qT8[:, 2 * eo:2 * eo + 2,
                                    128 * j:128 * (j + 1)],
                                start=(eo == 0), stop=(eo == dk // 2 - 1),
                                perf_mode=DR)
                    else:
                        for eo in range(dk):
                            nc.tensor.matmul(
                                ps[:, 128 * t:128 * (t + 1)],
                                kT16[:, eo, 128 * kt:128 * (kt + 1)],
                                qT16[:, eo, :],
                                start=(eo == 0), stop=(eo == dk - 1))
                attnT = attnT_p.tile([128, 4, 128], FP8 if fp8 else BF16,
                                     tag="at8" if fp8 else "at16")
                nc.scalar.activation(
                    out=attnT.rearrange("p a b -> p (a b)")[:, :128 * nt],
                    in_=ps[:, :128 * nt],
                    func=mybir.ActivationFunctionType.Exp, scale=scale)
                if g == nb - 1:
                    mask = (masksT8_sb[:, j].rearrange("p a b -> p (a b)")
                            if fp8 else
                            masks016_sb.rearrange("p a b -> p (a b)"))
                    tgt = attnT.rearrange(
                        "p a b -> p (a b)")[:, 128 * (nt - 2):128 * nt]
                    nc.gpsimd.tensor_tensor(
                        tgt, tgt, mask, op=mybir.AluOpType.mult)
                attnTs.append((attnT, nt))

            # pass 2: rowsum/64 and z accumulation over k-tile pairs
            n_pairs = P // 2
            for g, (attnT, nt) in enumerate(attnTs):
                for i in range(nt // 2):
                    pr = 2 * g + i
                    kt = 4 * g + 2 * i
                    st = (pr == 0)
                    sp = (pr == n_pairs - 1)
                    if fp8:
                        lhs = attnT[:, 2 * i:2 * i + 2, :]
                        nc.tensor.matmul(ps_r, lhs, ones8, start=st, stop=sp,
                                         perf_mode=DR)
                        for n in range(dn):
                            nc.tensor.matmul(
                                ps_z[:, 512 * n:512 * (n + 1)], lhs,
                                x8_nat[:, kt:kt + 2, 512 * n:512 * (n + 1)],
                                start=st, stop=sp, perf_mode=DR)
                    else:
                        for u in range(2):
                            lhs = attnT[:, 2 * i + u, :]
                            st2 = st and u == 0
                            sp2 = sp and u == 1
                            nc.tensor.matmul(ps_r, lhs, ones16[:, 0, :],
                                             start=st2, stop=sp2)
                            for n in range(dn):
                                nc.tensor.matmul(
                                    ps_z[:, 512 * n:512 * (n + 1)], lhs,
                                    xb16_nat[:, kt + u,
                                             512 * n:512 * (n + 1)],
                                    start=st2, stop=sp2)

            recip = sm_p.tile([128, 1], F32, tag="rc")
            nc.vector.reciprocal(recip, ps_r[:, 0:1])   # = 64 / rowsum

            # z/64 -> bf16 -> DMA transpose -> zT16; Pool casts to fp8
            z16 = z_p.tile([128, d], BF16, tag="z16")
            nc.scalar.activation(out=z16, in_=ps_z,
                                 func=mybir.ActivationFunctionType.Copy,
                                 scale=ZSCALE)
            zT16 = z_p.tile([128, dk, 128], BF16, tag="zT16")
            nc.sync.dma_start_transpose(zT16, z16)
            if fp8:
                zT = z_p.tile([128, dk, 128], FP8, tag="zT8")
                nc.gpsimd.tensor_copy(
                    zT.rearrange("p a b -> p (a b)"),
                    zT16.rearrange("p a b -> p (a b)"))
            else:
                zT = zT16

            # output projection, scaled by 64/rowsum
            out_sb = out_p.tile([128, d], F32, tag="osb")
            for n in range(dn):
                ps_o = ps_op.tile([128, 512], F32, tag="po")
                if fp8:
                    for ko in range(dk // 2):
                        nc.tensor.matmul(
                            ps_o, zT[:, 2 * ko:2 * ko + 2, :],
                            wv8_sb[:, 2 * ko:2 * ko + 2,
                                   512 * n:512 * (n + 1)],
                            start=(ko == 0), stop=(ko == dk // 2 - 1),
                            perf_mode=DR)
                else:
                    for ko in range(dk):
                        nc.tensor.matmul(
                            ps_o, zT[:, ko, :],
                            wv16_sb[:, ko, 512 * n:512 * (n + 1)],
                            start=(ko == 0), stop=(ko == dk - 1))
                nc.scalar.activation(
                    out=out_sb[:, 512 * n:512 * (n + 1)], in_=ps_o,
                    func=mybir.ActivationFunctionType.Copy, scale=recip)
            nc.sync.dma_start(out=out_dram.ap()[j], in_=out_sb)

    return nc


_COMPILED = {}


def _get_program(loop_n=1):
    key = f"v2_loop{loop_n}"

    if key not in _COMPILED:
        nc = bacc.Bacc("TRN2", target_bir_lowering=False, debug=False,
                       num_devices=N_CORES)
        build_attention_program(nc, loop_n=loop_n)
        nc.compile()
        _COMPILED[key] = nc
    return _COMPILED[key]


def _make_in_maps(x, Wq, Wk, Wv):
    import ml_dtypes
    bf = ml_dtypes.bfloat16
    f8 = ml_dtypes.float8_e4m3
    in_maps = []
    plans = []
    Wq16, Wk16, Wv16 = Wq.astype(bf), Wk.astype(bf), Wv.astype(bf)
    Wq8, Wk8, Wv8 = Wq.astype(f8), Wk.astype(f8), Wv.astype(f8)
    for c in range(N_CORES):
        b, h = divmod(c, 2)
        tiles = core_slot_tiles(h)
        plans.append((b, tiles))
        xb = x[b]                                             # [S, D] f32
        xbT = xb.T
        own_rows = np.concatenate(
            [xb[128 * qi:128 * qi + 128] for qi in tiles], axis=0)
        q0 = tiles[0]
        masksT = build_masksT(tiles)
        in_maps.append({
            "x8": xb.astype(f8),
            "xb16": np.ascontiguousarray(xb[:256]).astype(bf),
            "xT8": np.ascontiguousarray(xbT).astype(f8),
            "xT16": np.ascontiguousarray(xbT[:, :256]).astype(bf),
            "xqT8": np.ascontiguousarray(own_rows.T).astype(f8),
            "xq0T16": np.ascontiguousarray(
                xb[128 * q0:128 * q0 + 128].T).astype(bf),
            "Wq8": Wq8, "Wk8": Wk8, "Wv8": Wv8,
            "Wq16": Wq16, "Wk16": Wk16, "Wv16": Wv16,
            "masksT8": masksT.astype(f8),
            "masks016": masksT[0].astype(bf),
        })
    return in_maps, plans


def kernel(x, Wq, Wk, Wv):
    x = np.asarray(x, dtype=np.float32)
    Wq = np.asarray(Wq, dtype=np.float32)
    Wk = np.asarray(Wk, dtype=np.float32)
    Wv = np.asarray(Wv, dtype=np.float32)

    nc = _get_program()
    in_maps, plans = _make_in_maps(x, Wq, Wk, Wv)
    r = run_bass_kernel_spmd(nc, in_maps, list(range(N_CORES)))

    out = np.empty((B, S, D), dtype=np.float32)
    for c in range(N_CORES):
        b, tiles = plans[c]
        res = r.results[c]["out"]                             # [16, 128, D]
        for j, qi in enumerate(tiles):
            out[b, 128 * qi:128 * qi + 128] = res[j]
    return out


def _make_runner(nc):
    """One-bass_exec shard_map runner for `nc` (hook-compatible)."""
    import jax
    from jax.sharding import Mesh, PartitionSpec
    from jax.experimental.shard_map import shard_map
    from concourse.bass2jax import (_bass_exec_p, install_neuronx_cc_hook,
                                    partition_id_tensor)
    from concourse import mybir as _mb

    install_neuronx_cc_hook()
    partition_name = (nc.partition_id_tensor.name
                      if nc.partition_id_tensor else None)
    in_names, out_names, out_avals, zero_outs = [], [], [], []
    for alloc in nc.m.functions[0].allocations:
        if not isinstance(alloc, _mb.MemoryLocationSet):
            continue
        name = alloc.memorylocations[0].name
        if alloc.kind == "ExternalInput":
            if name != partition_name:
                in_names.append(name)
        elif alloc.kind == "ExternalOutput":
            shape = tuple(alloc.tensor_shape)
            dtype = _mb.dt.np(alloc.dtype)
            out_names.append(name)
            out_avals.append(jax.core.ShapedArray(shape, dtype))
            zero_outs.append(np.zeros(shape, dtype))
    n_params = len(in_names)
    bind_in_names = tuple(in_names + out_names +
                          ([partition_name] if partition_name else []))

    def _body(*args):
        extra = [partition_id_tensor()] if partition_name else []
        outs = _bass_exec_p.bind(
            *args, *extra,
            out_avals=tuple(out_avals),
            in_names=bind_in_names,
            out_names=tuple(out_names),
            lowering_input_output_aliases=(),
            sim_require_finite=True,
            sim_require_nnan=True,
            nc=nc)
        return tuple(outs)

    devices = jax.devices()[:N_CORES]
    mesh = Mesh(np.asarray(devices), ("core",))
    nin = n_params + len(out_names)
    fn = jax.jit(shard_map(
        _body, mesh=mesh,
        in_specs=(PartitionSpec("core"),) * nin,
        out_specs=(PartitionSpec("core"),) * len(out_names),
        check_rep=False), keep_unused=True)
    return fn, in_names, zero_outs


def bench(x, Wq, Wk, Wv, iters=64, iters_lo=16, trials=6):
    """Amortized HW timing, same protocol as kernel.py baseline."""
    import time
    import jax

    x = np.asarray(x, dtype=np.float32)
    in_maps, plans = _make_in_maps(
        x, np.asarray(Wq, np.float32), np.asarray(Wk, np.float32),
        np.asarray(Wv, np.float32))

    nc1 = _get_program()
    ncL = _get_program(loop_n=iters_lo)
    ncN = _get_program(loop_n=iters)

    f1, in_names, zero_outs = _make_runner(nc1)
    fL, _, _ = _make_runner(ncL)
    fN, _, _ = _make_runner(ncN)

    per_core = [[np.asarray(m[n]) for n in in_names] for m in in_maps]
    concat_in = [np.concatenate([per_core[c][i] for c in range(N_CORES)],
                                axis=0) for i in range(len(in_names))]
    concat_zo = [np.concatenate([z] * N_CORES, axis=0) for z in zero_outs]
    args = [jax.device_put(a) for a in concat_in + concat_zo]

    out1 = f1(*args); jax.block_until_ready(out1)
    outL = fL(*args); jax.block_until_ready(outL)
    outN = fN(*args); jax.block_until_ready(outN)

    tl = tn = float("inf")
    for _ in range(trials):
        t0 = time.perf_counter()
        r = fL(*args); jax.block_until_ready(r)
        tl = min(tl, time.perf_counter() - t0)
        t0 = time.perf_counter()
        r = fN(*args); jax.block_until_ready(r)
        tn = min(tn, time.perf_counter() - t0)
    per_iter_ns = (tn - tl) / (iters - iters_lo) * 1e9
    print(f"[bench] T{iters_lo}={tl*1e3:.2f} ms  T{iters}={tn*1e3:.2f} ms  "
          f"-> per-iter {per_iter_ns*1e-3:.1f} us", flush=True)

    d1 = np.asarray(out1[0])
    dN = np.asarray(outN[0])
    if not np.array_equal(d1, dN):
        print(f"[bench] WARNING loop/plain outputs differ "
              f"maxabs={np.abs(d1 - dN).max():.3e}", flush=True)

    outs_np = d1.reshape(N_CORES, len(core_slot_tiles(0)), 128, D)
    out = np.empty((B, S, D), dtype=np.float32)
    for c in range(N_CORES):
        b, tiles = plans[c]
        for j, qi in enumerate(tiles):
            out[b, 128 * qi:128 * qi + 128] = outs_np[c, j]
    return per_iter_ns, out


# revision 3
# speedup vs baseline: 4.3721x; 4.3721x over previous
"""Causal self-attention (single head) on 8 trn2 NeuronCores — fp8 edition.

Full inputs:  x [4, 4096, 1024] f32, Wq/Wk/Wv [1024, 1024] f32.
Output:       [4, 4096, 1024] f32 = softmax(causal(q k^T / sqrt(d))) @ v.

Sharding: 2 cores per batch; query tiles split qi%4 in {0,3} vs {1,2}
(exactly balanced causal work), slot j covers P=2j+2 key tiles.

v2: all matmuls in fp8-e4m3 with DoubleRow perf mode (2 k-tiles of
contraction per pass, ~4x bf16 throughput measured), EXCEPT slot 0
(queries attending to <=256 keys) which runs in bf16: absolute output
error for rows with few attended keys is dominated by input
quantization noise that does not average out, so slot 0 keeps bf16
inputs end-to-end.

Scores are computed TRANSPOSED (sT[k, q] per 128x128 k-tile block):
lhsT = kT-block, rhs = qT-slot. exp() then writes attnT directly (no
transposes on the attention path). Causal masking is multiplicative
(0/1) on attnT after exp. Row sums (needed for softmax normalization)
come from a tiny extra matmul per k-tile pair: lhsT=attnT, rhs=ones/64
-> psum [128q, 1] accumulating rowsum/64; reciprocal gives 64/rowsum,
z is scaled by 1/64 into fp8 (avoids e4m3 overflow: |z| can reach
~300 > 240 = e4m3 max), and the two scales cancel exactly in the
output projection's recip-scaled copy.

PV is reassociated: out = ((attn @ x) @ Wv) / rowsum, so V is never
materialized. q/k magnitudes stay natural (std ~0.58, e4m3 sweet
spot); the 1/sqrt(d) softmax scale is folded into the Exp activation.
"""

import math
import os
import numpy as np
from contextlib import ExitStack

import concourse.bass as bass
import concourse.tile as tile
from concourse import bacc, mybir
from concourse.bass_utils import run_bass_kernel_spmd

F32 = mybir.dt.float32
BF16 = mybir.dt.bfloat16
FP8 = mybir.dt.float8e4
DR = mybir.MatmulPerfMode.DoubleRow

B = 4
S = 4096
D = 1024
N_CORES = 8
ZSCALE = 1.0 / 64.0          # keeps |z| and rowsum within fp8/clean f32


def core_slot_tiles(h: int) -> list[int]:
    """Query-tile index (qi) handled in slot j, for core half h."""
    out = []
    for m in range(8):
        if h == 0:
            out += [4 * m, 4 * m + 3]
        else:
            out += [4 * m + 1, 4 * m + 2]
    return out


def build_masksT(tiles: list[int]) -> np.ndarray:
    """[n_slots, 2, 128, 128] multiplicative masks (k-major) for the last
    two k-tiles of each slot: m[t, k, q] = 1 if key_global <= query_global."""
    n_slots = len(tiles)
    masks = np.zeros((n_slots, 2, 128, 128), dtype=np.float32)
    k = np.arange(128)[:, None]
    q = np.arange(128)[None, :]
    for j, qi in enumerate(tiles):
        P = 2 * j + 2
        for t in range(2):
            kt = P - 2 + t
            valid = (128 * kt + k) <= (128 * qi + q)
            masks[j, t] = valid.astype(np.float32)
    return masks


def build_attention_program(nc, loop_n: int = 1):
    s_tiles = S // 128              # 32
    n_slots = s_tiles // 2          # 16
    d = D
    dk = d // 128                   # 8 contraction chunks of 128
    dn = d // 512                   # 2 output column blocks
    sq = n_slots * 128              # 2048 own query rows
    scale = 1.0 / math.sqrt(float(d))

    x8_in = nc.dram_tensor("x8", [S, d], FP8, kind="ExternalInput")
    xb16_in = nc.dram_tensor("xb16", [256, d], BF16, kind="ExternalInput")
    xT8_in = nc.dram_tensor("xT8", [d, S], FP8, kind="ExternalInput")
    xT16_in = nc.dram_tensor("xT16", [d, 256], BF16, kind="ExternalInput")
    xqT8_in = nc.dram_tensor("xqT8", [d, sq], FP8, kind="ExternalInput")
    xq0T16_in = nc.dram_tensor("xq0T16", [d, 128], BF16, kind="ExternalInput")
    wq8_in = nc.dram_tensor("Wq8", [d, d], FP8, kind="ExternalInput")
    wk8_in = nc.dram_tensor("Wk8", [d, d], FP8, kind="ExternalInput")
    wv8_in = nc.dram_tensor("Wv8", [d, d], FP8, kind="ExternalInput")
    wq16_in = nc.dram_tensor("Wq16", [d, d], BF16, kind="ExternalInput")
    wk16_in = nc.dram_tensor("Wk16", [d, d], BF16, kind="ExternalInput")
    wv16_in = nc.dram_tensor("Wv16", [d, d], BF16, kind="ExternalInput")
    masksT8_in = nc.dram_tensor("masksT8", [n_slots, 2, 128, 128], FP8,
                                kind="ExternalInput")
    masks016_in = nc.dram_tensor("masks016", [2, 128, 128], BF16,
                                 kind="ExternalInput")
    out_dram = nc.dram_tensor("out", [n_slots, 128, d], F32,
                              kind="ExternalOutput")

    x8_r = x8_in.ap().rearrange("(t p) d -> p t d", p=128)     # [128,32,1024]
    xb16_r = xb16_in.ap().rearrange("(t p) d -> p t d", p=128)  # [128,2,1024]
    xT8_r = xT8_in.ap().rearrange("(do p) s -> p do s", p=128)
    xT16_r = xT16_in.ap().rearrange("(do p) s -> p do s", p=128)
    xqT8_r = xqT8_in.ap().rearrange("(do p) s -> p do s", p=128)
    xq0T16_r = xq0T16_in.ap().rearrange("(do p) s -> p do s", p=128)
    wq8_r = wq8_in.ap().rearrange("(ko p) n -> p ko n", p=128)
    wk8_r = wk8_in.ap().rearrange("(ko p) n -> p ko n", p=128)
    wv8_r = wv8_in.ap().rearrange("(ko p) n -> p ko n", p=128)
    wq16_r = wq16_in.ap().rearrange("(ko p) n -> p ko n", p=128)
    wk16_r = wk16_in.ap().rearrange("(ko p) n -> p ko n", p=128)
    wv16_r = wv16_in.ap().rearrange("(ko p) n -> p ko n", p=128)
    masksT8_r = masksT8_in.ap().rearrange("j t k q -> k j t q")
    masks016_r = masks016_in.ap().rearrange("t k q -> k t q")

    with tile.TileContext(nc) as tc, ExitStack() as outer, \
         ExitStack() as ctx:
        if loop_n > 1:
            outer.enter_context(tc.For_i(0, loop_n, 1))
        res = ctx.enter_context(tc.tile_pool(name="res", bufs=1))
        x8_nat = res.tile([128, s_tiles, d], FP8)
        xb16_nat = res.tile([128, 2, d], BF16)
        kT8 = res.tile([128, dk, S], FP8)
        kT16 = res.tile([128, dk, 256], BF16)
        qT8 = res.tile([128, dk, sq], FP8)
        qT16 = res.tile([128, dk, 128], BF16)
        wv8_sb = res.tile([128, dk, d], FP8)
        wv16_sb = res.tile([128, dk, d], BF16)
        masksT8_sb = res.tile([128, n_slots, 2, 128], FP8)
        masks016_sb = res.tile([128, 2, 128], BF16)
        rs_w = 1
        ones8 = res.tile([128, 2, rs_w], FP8)
        ones16 = res.tile([128, 2, rs_w], BF16)
        nc.vector.memset(ones8, ZSCALE)
        nc.vector.memset(ones16, ZSCALE)

        def load_residents():
            """Stage-2 residents on the Activation HWDGE ring so stage-1
            input chunks on the SP ring are never stuck behind them."""
            eng = (nc.scalar if os.environ.get("V2_RES_ACT", "1") == "1"
                   else nc.sync)
            eng.dma_start(out=masks016_sb, in_=masks016_r)
            eng.dma_start(out=masksT8_sb, in_=masksT8_r)
            eng.dma_start(out=xb16_nat, in_=xb16_r)
            for piece in range(4):
                eng.dma_start(
                    out=x8_nat[:, 8 * piece:8 * (piece + 1), :],
                    in_=x8_r[:, 8 * piece:8 * (piece + 1), :])
            eng.dma_start(out=wv16_sb, in_=wv16_r)
            eng.dma_start(out=wv8_sb, in_=wv8_r)

        # ---------------- stage 1: projections ----------------
        stage1 = ExitStack()
        ps_proj = stage1.enter_context(
            tc.tile_pool(name="ps_proj", bufs=4, space="PSUM"))

        def copy_out(eng, dst, src):
            if eng is nc.scalar:
                nc.scalar.activation(
                    out=dst, in_=src,
                    func=mybir.ActivationFunctionType.Copy)
            else:
                eng.tensor_copy(dst, src)

        def proj8(dst, w_sb, src_r, n_chunks, copy_engines):
            """dst[:, dot, 512ci:+512] = W^T @ src for fp8 DoubleRow."""
            with tc.tile_pool(name="xt8", bufs=2) as xtp:
                for ci in range(n_chunks):
                    xc = xtp.tile([128, dk, 512], FP8, tag="x8c")
                    nc.sync.dma_start(
                        out=xc, in_=src_r[:, :, 512 * ci:512 * (ci + 1)])
                    for dot in range(dk):
                        ps = ps_proj.tile([128, 512], F32, tag="pp")
                        for ko in range(dk // 2):
                            nc.tensor.matmul(
                                ps,
                                w_sb[:, 2 * ko:2 * ko + 2,
                                     128 * dot:128 * (dot + 1)],
                                xc[:, 2 * ko:2 * ko + 2, :],
                                start=(ko == 0), stop=(ko == dk // 2 - 1),
                                perf_mode=DR)
                        eng = copy_engines[(ci * dk + dot) % len(copy_engines)]
                        copy_out(eng, dst[:, dot, 512 * ci:512 * (ci + 1)], ps)

        def proj16(dst, w_sb, src_r, width, copy_engines):
            """bf16 mini-projection of `width` columns (width <= 512)."""
            with tc.tile_pool(name="xt16", bufs=1) as xtp:
                xc = xtp.tile([128, dk, width], BF16, tag="x16c")
                nc.sync.dma_start(out=xc, in_=src_r)
                for dot in range(dk):
                    ps = ps_proj.tile([128, 512], F32, tag="pp")
                    for ko in range(dk):
                        nc.tensor.matmul(
                            ps[:, :width],
                            w_sb[:, ko, 128 * dot:128 * (dot + 1)],
                            xc[:, ko, :],
                            start=(ko == 0), stop=(ko == dk - 1))
                    eng = copy_engines[dot % len(copy_engines)]
                    copy_out(eng, dst[:, dot, :], ps[:, :width])

        both = [nc.vector, nc.scalar]
        with tc.tile_pool(name="wk", bufs=1) as wkp:
            wk8_sb = wkp.tile([128, dk, d], FP8, tag="w8")
            nc.sync.dma_start(out=wk8_sb, in_=wk8_r)
            proj8(kT8, wk8_sb, xT8_r, S // 512, both)
            wk16_sb = wkp.tile([128, dk, d], BF16, tag="w16")
            nc.sync.dma_start(out=wk16_sb, in_=wk16_r)
            proj16(kT16, wk16_sb, xT16_r, 256, both)
        with tc.tile_pool(name="wq", bufs=1) as wqp:
            wq8_sb = wqp.tile([128, dk, d], FP8, tag="w8")
            nc.sync.dma_start(out=wq8_sb, in_=wq8_r)
            proj8(qT8, wq8_sb, xqT8_r, sq // 512, both)
            wq16_sb = wqp.tile([128, dk, d], BF16, tag="w16")
            nc.sync.dma_start(out=wq16_sb, in_=wq16_r)
            proj16(qT16, wq16_sb, xq0T16_r, 128, both)
        if resmode != "top":
            load_residents()

        # ---------------- stage 2: attention ----------------
        stage1.close()
        resmode = "late"
        def load_residents():
            if resmode == "top":
                nc.sync.dma_start(out=x8_nat, in_=x8_r)
                nc.sync.dma_start(out=xb16_nat, in_=xb16_r)
                nc.sync.dma_start(out=wv8_sb, in_=wv8_r)
                nc.sync.dma_start(out=wv16_sb, in_=wv16_r)
                nc.sync.dma_start(out=masksT8_sb, in_=masksT8_r)
                nc.sync.dma_start(out=masks016_sb, in_=masks016_r)
            else:
                nc.sync.dma_start(out=masks016_sb, in_=masks016_r)
                nc.sync.dma_start(out=masksT8_sb, in_=masksT8_r)
                nc.sync.dma_start(out=xb16_nat, in_=xb16_r)
                for piece in range(4):
                    nc.sync.dma_start(
                        out=x8_nat[:, 8 * piece:8 * (piece + 1), :],
                        in_=x8_r[:, 8 * piece:8 * (piece + 1), :])
                nc.sync.dma_start(out=wv16_sb, in_=wv16_r)
                nc.sync.dma_start(out=wv8_sb, in_=wv8_r)
        if resmode == "top":
            load_residents()

        # ---------------- stage 1: projections ----------------
        stage1 = ExitStack()
        ps_proj = stage1.enter_context(
            tc.tile_pool(name="ps_proj", bufs=4, space="PSUM"))

        def copy_out(eng, dst, src):
            if eng is nc.scalar:
                nc.scalar.activation(
                    out=dst, in_=src,
                    func=mybir.ActivationFunctionType.Copy)
            else:
                eng.tensor_copy(dst, src)

        def proj8(dst, w_sb, src_r, n_chunks, copy_engines):
            """dst[:, dot, 512ci:+512] = W^T @ src for fp8 DoubleRow."""
            with tc.tile_pool(name="xt8", bufs=2) as xtp:
                for ci in range(n_chunks):
                    xc = xtp.tile([128, dk, 512], FP8, tag="x8c")
                    nc.sync.dma_start(
                        out=xc, in_=src_r[:, :, 512 * ci:512 * (ci + 1)])
                    for dot in range(dk):
                        ps = ps_proj.tile([128, 512], F32, tag="pp")
                        for ko in range(dk // 2):
                            nc.tensor.matmul(
                                ps,
                                w_sb[:, 2 * ko:2 * ko + 2,
                                     128 * dot:128 * (dot + 1)],
                                xc[:, 2 * ko:2 * ko + 2, :],
                                start=(ko == 0), stop=(ko == dk // 2 - 1),
                                perf_mode=DR)
                        eng = copy_engines[(ci * dk + dot) % len(copy_engines)]
                        copy_out(eng, dst[:, dot, 512 * ci:512 * (ci + 1)], ps)

        def proj16(dst, w_sb, src_r, width, copy_engines):
            """bf16 mini-projection of `width` columns (width <= 512)."""
            with tc.tile_pool(name="xt16", bufs=1) as xtp:
                xc = xtp.tile([128, dk, width], BF16, tag="x16c")
                nc.sync.dma_start(out=xc, in_=src_r)
                for dot in range(dk):
                    ps = ps_proj.tile([128, 512], F32, tag="pp")
                    for ko in range(dk):
                        nc.tensor.matmul(
                            ps[:, :width],
                            w_sb[:, ko, 128 * dot:128 * (dot + 1)],
                            xc[:, ko, :],
                            start=(ko == 0), stop=(ko == dk - 1))
                    eng = copy_engines[dot % len(copy_engines)]
                    copy_out(eng, dst[:, dot, :], ps[:, :width])

        both = [nc.vector, nc.scalar]
        with tc.tile_pool(name="wk", bufs=1) as wkp:
            wk8_sb = wkp.tile([128, dk, d], FP8, tag="w8")
            nc.sync.dma_start(out=wk8_sb, in_=wk8_r)
            proj8(kT8, wk8_sb, xT8_r, S // 512, both)
            wk16_sb = wkp.tile([128, dk, d], BF16, tag="w16")
            nc.sync.dma_start(out=wk16_sb, in_=wk16_r)
            proj16(kT16, wk16_sb, xT16_r, 256, both)
        with tc.tile_pool(name="wq", bufs=1) as wqp:
            wq8_sb = wqp.tile([128, dk, d], FP8, tag="w8")
            nc.sync.dma_start(out=wq8_sb, in_=wq8_r)
            proj8(qT8, wq8_sb, xqT8_r, sq // 512, both)
            wq16_sb = wqp.tile([128, dk, d], BF16, tag="w16")
            nc.sync.dma_start(out=wq16_sb, in_=wq16_r)
            proj16(qT16, wq16_sb, xq0T16_r, 128, both)
        if resmode != "top":
            load_residents()

        # ---------------- stage 2: attention ----------------
        stage1.close()
        ps_s = ctx.enter_context(
            tc.tile_pool(name="ps_s", bufs=3, space="PSUM"))
        ps_zp = ctx.enter_context(
            tc.tile_pool(name="ps_z", bufs=1, space="PSUM"))
        ps_op = ctx.enter_context(
            tc.tile_pool(name="ps_o", bufs=2, space="PSUM"))
        attnT_p = ctx.enter_context(tc.tile_pool(name="attnT_p", bufs=6))
        z_p = ctx.enter_context(tc.tile_pool(name="z_p", bufs=2))
        sm_p = ctx.enter_context(tc.tile_pool(name="sm_p", bufs=2))
        out_p = ctx.enter_context(tc.tile_pool(name="out_p", bufs=2))

        for j in range(n_slots):
            P = 2 * j + 2                    # k-tiles (128 keys each)
            nb = (P + 3) // 4                # psum blocks of 4 k-tiles
            fp8 = j >= 1
            ps_zr = ps_zp.tile([128, d + 512], F32, tag="z")
            ps_z = ps_zr[:, :d]
            ps_r = ps_zr[:, d:d + rs_w]

            attnTs = []
            # pass 1: scoresT -> exp -> (masked) attnT, per 4-k-tile block
            for g in range(nb):
                nt = 4 if (g < nb - 1 or P % 4 == 0) else P % 4
                ps = ps_s.tile([128, 512], F32, tag="ps")
                for t in range(nt):
                    kt = 4 * g + t
                    if fp8:
                        for eo in range(dk // 2):
                            nc.tensor.matmul(
                                ps[:, 128 * t:128 * (t + 1)],
                                kT8[:, 2 * eo:2 * eo + 2,
                                    128 * kt:128 * (kt + 1)],
                                qT8[:, 2 * eo:2 * eo + 2,
                                    128 * j:128 * (j + 1)],
                                start=(eo == 0), stop=(eo == dk // 2 - 1),
                                perf_mode=DR)
                    else:
                        for eo in range(dk):
                            nc.tensor.matmul(
                                ps[:, 128 * t:128 * (t + 1)],
                                kT16[:, eo, 128 * kt:128 * (kt + 1)],
                                qT16[:, eo, :],
                                start=(eo == 0), stop=(eo == dk - 1))
                attnT = attnT_p.tile([128, 4, 128], FP8 if fp8 else BF16,
                                     tag="at8" if fp8 else "at16")
                nc.scalar.activation(
                    out=attnT.rearrange("p a b -> p (a b)")[:, :128 * nt],
                    in_=ps[:, :128 * nt],
                    func=mybir.ActivationFunctionType.Exp, scale=scale)
                if g == nb - 1:
                    mask = (masksT8_sb[:, j].rearrange("p a b -> p (a b)")
                            if fp8 else
                            masks016_sb.rearrange("p a b -> p (a b)"))
                    tgt = attnT.rearrange(
                        "p a b -> p (a b)")[:, 128 * (nt - 2):128 * nt]
                    nc.vector.tensor_tensor(
                        tgt, tgt, mask, op=mybir.AluOpType.mult)
                attnTs.append((attnT, nt))

            # pass 2: rowsum/64 and z accumulation over k-tile pairs
            n_pairs = P // 2
            for g, (attnT, nt) in enumerate(attnTs):
                for i in range(nt // 2):
                    pr = 2 * g + i
                    kt = 4 * g + 2 * i
                    st = (pr == 0)
                    sp = (pr == n_pairs - 1)
                    if fp8:
                        lhs = attnT[:, 2 * i:2 * i + 2, :]
                        nc.tensor.matmul(ps_r, lhs, ones8, start=st, stop=sp,
                                         perf_mode=DR)
                        for n in range(dn):
                            nc.tensor.matmul(
                                ps_z[:, 512 * n:512 * (n + 1)], lhs,
                                x8_nat[:, kt:kt + 2, 512 * n:512 * (n + 1)],
                                start=st, stop=sp, perf_mode=DR)
                    else:
                        for u in range(2):
                            lhs = attnT[:, 2 * i + u, :]
                            st2 = st and u == 0
                            sp2 = sp and u == 1
                            nc.tensor.matmul(ps_r, lhs, ones16[:, 0, :],
                                             start=st2, stop=sp2)
                            for n in range(dn):
                                nc.tensor.matmul(
                                    ps_z[:, 512 * n:512 * (n + 1)], lhs,
                                    xb16_nat[:, kt + u,
                                             512 * n:512 * (n + 1)],
                                    start=st2, stop=sp2)

            recip = sm_p.tile([128, 1], F32, tag="rc")
            nc.vector.reciprocal(recip, ps_r[:, 0:1])   # = 64 / rowsum

            # z/64 -> bf16 -> DMA transpose -> zT16; Pool casts to fp8
            z16 = z_p.tile([128, d], BF16, tag="z16")
            nc.scalar.activation(out=z16, in_=ps_z,
                                 func=mybir.ActivationFunctionType.Copy,
                                 scale=ZSCALE)
            zT16 = z_p.tile([128, dk, 128], BF16, tag="zT16")
            nc.sync.dma_start_transpose(zT16, z16)
            if fp8:
                zT = z_p.tile([128, dk, 128], FP8, tag="zT8")
                nc.vector.tensor_copy(
                    zT.rearrange("p a b -> p (a b)"),
                    zT16.rearrange("p a b -> p (a b)"))
            else:
                zT = zT16

            # output projection, scaled by 64/rowsum
            out_sb = out_p.tile([128, d], F32, tag="osb")
            for n in range(dn):
                ps_o = ps_op.tile([128, 512], F32, tag="po")
                if fp8:
                    for ko in range(dk // 2):
                        nc.tensor.matmul(
                            ps_o, zT[:, 2 * ko:2 * ko + 2, :],
                            wv8_sb[:, 2 * ko:2 * ko + 2,
                                   512 * n:512 * (n + 1)],
                            start=(ko == 0), stop=(ko == dk // 2 - 1),
                            perf_mode=DR)
                else:
                    for ko in range(dk):
                        nc.tensor.matmul(
                            ps_o, zT[:, ko, :],
                            wv16_sb[:, ko, 512 * n:512 * (n + 1)],
                            start=(ko == 0), stop=(ko == dk - 1))
                nc.scalar.activation(
                    out=out_sb[:, 512 * n:512 * (n + 1)], in_=ps_o,
                    func=mybir.ActivationFunctionType.Copy, scale=recip)
            nc.sync.dma_start(out=out_dram.ap()[j], in_=out_sb)

    return nc


_COMPILED = {}


def _get_program(loop_n=1):
    key = f"v2_loop{loop_n}"

    if key not in _COMPILED:
        nc = bacc.Bacc("TRN2", target_bir_lowering=False, debug=False,
                       num_devices=N_CORES)
        build_attention_program(nc, loop_n=loop_n)
        nc.compile()
        _COMPILED[key] = nc
    return _COMPILED[key]


def _make_in_maps(x, Wq, Wk, Wv):
    import ml_dtypes
    bf = ml_dtypes.bfloat16
    f8 = ml_dtypes.float8_e4m3
    in_maps = []
    plans = []
    Wq16, Wk16, Wv16 = Wq.astype(bf), Wk.astype(bf), Wv.astype(bf)
    Wq8, Wk8, Wv8 = Wq.astype(f8), Wk.astype(f8), Wv.astype(f8)
    for c in range(N_CORES):
        b, h = divmod(c, 2)
        tiles = core_slot_tiles(h)
        plans.append((b, tiles))
        xb = x[b]                                             # [S, D] f32
        xbT = xb.T
        own_rows = np.concatenate(
            [xb[128 * qi:128 * qi + 128] for qi in tiles], axis=0)
        q0 = tiles[0]
        masksT = build_masksT(tiles)
        in_maps.append({
            "x8": xb.astype(f8),
            "xb16": np.ascontiguousarray(xb[:256]).astype(bf),
            "xT8": np.ascontiguousarray(xbT).astype(f8),
            "xT16": np.ascontiguousarray(xbT[:, :256]).astype(bf),
            "xqT8": np.ascontiguousarray(own_rows.T).astype(f8),
            "xq0T16": np.ascontiguousarray(
                xb[128 * q0:128 * q0 + 128].T).astype(bf),
            "Wq8": Wq8, "Wk8": Wk8, "Wv8": Wv8,
            "Wq16": Wq16, "Wk16": Wk16, "Wv16": Wv16,
            "masksT8": masksT.astype(f8),
            "masks016": masksT[0].astype(bf),
        })
    return in_maps, plans


def kernel(x, Wq, Wk, Wv):
    x = np.asarray(x, dtype=np.float32)
    Wq = np.asarray(Wq, dtype=np.float32)
    Wk = np.asarray(Wk, dtype=np.float32)
    Wv = np.asarray(Wv, dtype=np.float32)

    nc = _get_program()
    in_maps, plans = _make_in_maps(x, Wq, Wk, Wv)
    r = run_bass_kernel_spmd(nc, in_maps, list(range(N_CORES)))

    out = np.empty((B, S, D), dtype=np.float32)
    for c in range(N_CORES):
        b, tiles = plans[c]
        res = r.results[c]["out"]                             # [16, 128, D]
        for j, qi in enumerate(tiles):
            out[b, 128 * qi:128 * qi + 128] = res[j]
    return out


def _make_runner(nc):
    """One-bass_exec shard_map runner for `nc` (hook-compatible)."""
    import jax
    from jax.sharding import Mesh, PartitionSpec
    from jax.experimental.shard_map import shard_map
    from concourse.bass2jax import (_bass_exec_p, install_neuronx_cc_hook,
                                    partition_id_tensor)
    from concourse import mybir as _mb

    install_neuronx_cc_hook()
    partition_name = (nc.partition_id_tensor.name
                      if nc.partition_id_tensor else None)
    in_names, out_names, out_avals, zero_outs = [], [], [], []
    for alloc in nc.m.functions[0].allocations:
        if not isinstance(alloc, _mb.MemoryLocationSet):
            continue
        name = alloc.memorylocations[0].name
        if alloc.kind == "ExternalInput":
            if name != partition_name:
                in_names.append(name)
        elif alloc.kind == "ExternalOutput":
            shape = tuple(alloc.tensor_shape)
            dtype = _mb.dt.np(alloc.dtype)
            out_names.append(name)
            out_avals.append(jax.core.ShapedArray(shape, dtype))
            zero_outs.append(np.zeros(shape, dtype))
    n_params = len(in_names)
    bind_in_names = tuple(in_names + out_names +
                          ([partition_name] if partition_name else []))

    def _body(*args):
        extra = [partition_id_tensor()] if partition_name else []
        outs = _bass_exec_p.bind(
            *args, *extra,
            out_avals=tuple(out_avals),
            in_names=bind_in_names,
            out_names=tuple(out_names),
            lowering_input_output_aliases=(),
            sim_require_finite=True,
            sim_require_nnan=True,
            nc=nc)
        return tuple(outs)

    devices = jax.devices()[:N_CORES]
    mesh = Mesh(np.asarray(devices), ("core",))
    nin = n_params + len(out_names)
    fn = jax.jit(shard_map(
        _body, mesh=mesh,
        in_specs=(PartitionSpec("core"),) * nin,
        out_specs=(PartitionSpec("core"),) * len(out_names),
        check_rep=False), keep_unused=True)
    return fn, in_names, zero_outs


def bench(x, Wq, Wk, Wv, iters=64, iters_lo=16, trials=6):
    """Amortized HW timing, same protocol as kernel.py baseline."""
    import time
    import jax

    x = np.asarray(x, dtype=np.float32)
    in_maps, plans = _make_in_maps(
        x, np.asarray(Wq, np.float32), np.asarray(Wk, np.float32),
        np.asarray(Wv, np.float32))

    nc1 = _get_program()
    ncL = _get_program(loop_n=iters_lo)
    ncN = _get_program(loop_n=iters)

    f1, in_names, zero_outs = _make_runner(nc1)
    fL, _, _ = _make_runner(ncL)
    fN, _, _ = _make_runner(ncN)

    per_core = [[np.asarray(m[n]) for n in in_names] for m in in_maps]
    concat_in = [np.concatenate([per_core[c][i] for c in range(N_CORES)],
                                axis=0) for i in range(len(in_names))]
    concat_zo = [np.concatenate([z] * N_CORES, axis=0) for z in zero_outs]
    args = [jax.device_put(a) for a in concat_in + concat_zo]

    out1 = f1(*args); jax.block_until_ready(out1)
    outL = fL(*args); jax.block_until_ready(outL)
    outN = fN(*args); jax.block_until_ready(outN)

    tl = tn = float("inf")
    for _ in range(trials):
        t0 = time.perf_counter()
        r = fL(*args); jax.block_until_ready(r)
        tl = min(tl, time.perf_counter() - t0)
        t0 = time.perf_counter()
        r = fN(*args); jax.block_until_ready(r)
        tn = min(tn, time.perf_counter() - t0)
    per_iter_ns = (tn - tl) / (iters - iters_lo) * 1e9
    print(f"[bench] T{iters_lo}={tl*1e3:.2f} ms  T{iters}={tn*1e3:.2f} ms  "
          f"-> per-iter {per_iter_ns*1e-3:.1f} us", flush=True)

    d1 = np.asarray(out1[0])
    dN = np.asarray(outN[0])
    if not np.array_equal(d1, dN):
        print(f"[bench] WARNING loop/plain outputs differ "
              f"maxabs={np.abs(d1 - dN).max():.3e}", flush=True)

    outs_np = d1.reshape(N_CORES, len(core_slot_tiles(0)), 128, D)
    out = np.empty((B, S, D), dtype=np.float32)
    for c in range(N_CORES):
        b, tiles = plans[c]
        for j, qi in enumerate(tiles):
            out[b, 128 * qi:128 * qi + 128] = outs_np[c, j]
    return per_iter_ns, out
